# revision 5
# baseline (speedup 1.0000x reference)
"""Trainium2 Bass kernel for a dense transformer encoder layer.

Problem: B=1, S=4096, D=512, F=2048, H=8 heads (Dh=64), fp32 reference,
attention WITHOUT 1/sqrt(Dh) scaling, int mask (0 -> -1e9 before softmax),
two LayerNorms, ReLU FFN.

Sharding (query/row-parallel, no collectives): every core redundantly
computes the full kT = (x@wk).T and v = x@wv, plus its own 512-query
shard. Each core computes attention + output projection + LN + FFN + LN
for its queries and writes outT (D, 512); the host transposes and
concatenates the shards.

v2 structure (fused pipeline, all engines overlapped):
  - The kT/v production loop (8 t-blocks of 512) is FUSED with group-0
    attention (heads 0-3): as soon as a 512-key block's kT (feature
    chunks 0,1) and v land in SBUF, the scores/exp/mask/AV for its four
    128-key chunks run. kT chunks 2,3 (only needed by group 1) are
    emitted last in each block as PE stall-filler, so the PE never idles
    long enough for the HAM clock monitor to re-throttle it to 1.2 GHz.
  - ALL projection biases (bq/bk/bv) are folded in as K=1 seed matmuls
    into PSUM before the accumulation chain (216ns each on the PE), and
    PSUM->SBUF evacuation is done by DVE tensor_copy at 2x rate. The ACT
    engine does nothing but exp (its 1 elem/cycle/lane @1.2GHz is the
    attention-phase floor) + the two LN sqrt calls.
  - PSUM budget in the fused phase: 4 banks out_ps (heads 0-3) + 2 banks
    scores + 1 bank kT chain + 1 bank v chain = 8 exactly. Group 1 runs
    afterwards from SBUF with double-buffered score PSUM (ACT ~100%
    duty) while the wo/w1/w2 weights for phase 3 prefetch over DMA.
  - Phase 3 (out-proj + LN1 + FFN + LN2) then runs with zero DMA waits.

Softmax skips max-subtraction (|scores| < ~60 fits bf16 range); the
ones-column in v yields denominators for free; per-query 1/sum is folded
in via a K=1 broadcast matmul. LayerNorm runs transposed: partition-dim
statistics via ones-vector matmuls, per-column stats broadcast with K=1
outer products, gamma/beta folded into the broadcast.

dtypes: fp16 (10-bit mantissa) for QKV projections, K/Q storage, scores,
FFN/out-projection weights+activations; fp32r for K=1 broadcast matmuls
and LN/residual arithmetic; bf16 for exp outputs / V / mask; fp32
accumulation in PSUM.
"""

import numpy as np
import ml_dtypes

import concourse.bass as bass
import concourse.bacc as bacc
import concourse.tile as tile
from concourse import mybir
from concourse.bass import ts, ds
from concourse.bass_utils import run_bass_kernel_spmd

AF = mybir.ActivationFunctionType
F32 = mybir.dt.float32
DT = mybir.dt.float32r  # fp32 storage, single-pass PE mode
DT16 = mybir.dt.float16
BF16 = mybir.dt.bfloat16

N_CORES = 8
EPS = 1e-5


def build_encoder_kernel(nc, S=4096, D=512, F=2048, H=8, n_cores=8):
    """Emit the SPMD per-core program. Returns nothing (declares DRAM I/O)."""
    P = 128
    SH = S // n_cores          # query shard per core
    DC = D // P                # feature chunks of 128
    FC = F // P                # ffn chunks of 128
    TB = S // 512              # 512-wide t blocks
    TC = S // P                # 128-tall t chunks
    Dh = D // H
    assert Dh == 64 and DC * P == D and SH % 2 == 0

    d = lambda name, shape, dt: nc.dram_tensor(name, shape, dt, kind="ExternalInput").ap()
    xT = d("xT", [D, S], DT16)
    xsT = d("xsT", [D, SH], DT)
    xs16 = d("xs16", [D, SH], DT16)
    maskT = d("maskT", [S, SH], BF16)
    wq, wk, wv, wo = (d(n, [D, D], DT16) for n in ("wq", "wk", "wv", "wo"))
    w1 = d("w1", [D, F], DT16)
    w2 = d("w2", [F, D], DT16)
    bq, bk, bv = (d(n, [D], DT) for n in ("bq", "bk", "bv"))
    bo = d("bo", [D], F32)
    b1 = d("b1", [F], F32)
    b2 = d("b2", [D], F32)
    g1, be1, g2, be2 = (d(n, [D], DT) for n in ("g1", "be1", "g2", "be2"))
    ones = d("ones", [512], DT)
    outT = nc.dram_tensor("outT", [D, SH], F32, kind="ExternalOutput").ap()

    with tile.TileContext(nc) as tc:
        _emit(nc, tc, locals())


def _emit(nc, tc, io):
    P = 128
    xT, maskT, outT = io["xT"], io["maskT"], io["outT"]
    S, D, F, H = io["S"], io["D"], io["F"], io["H"]
    SH, DC, FC, TB, TC, Dh = io["SH"], io["DC"], io["FC"], io["TB"], io["TC"], io["Dh"]
    HPC = P // Dh              # heads per 128-feature chunk (2)

    from contextlib import ExitStack
    with ExitStack() as root:
        gconst = root.enter_context(tc.tile_pool(name="gconst", bufs=1))
        gbig = root.enter_context(tc.tile_pool(name="gbig", bufs=1))
        p3w = root.enter_context(tc.tile_pool(name="p3w", bufs=1))

        # ---- startup DMAs, highest priority first ----
        xs16_sb = gbig.tile([P, DC, SH], DT16)    # own x shard fp16 (q proj rhs)
        nc.sync.dma_start(out=xs16_sb, in_=io["xs16"].rearrange("(c p) s -> p c s", p=P))

        def load_row(name, dt=DT):                # (n,) -> [1, n] row
            t = gconst.tile([1, io[name].shape[0]], dt, tag=f"row_{name}", name=f"row_{name}")
            nc.sync.dma_start(out=t, in_=io[name][None, :])
            return t

        ones_row = gconst.tile([1, P], DT)        # lhsT for K=1 broadcasts
        nc.sync.dma_start(out=ones_row, in_=io["ones"][None, :P])
        ones_col = gconst.tile([P, 1], DT)        # lhsT for partition sums
        nc.sync.dma_start(out=ones_col, in_=io["ones"][:P, None])
        ones_s = gconst.tile([1, SH], DT)         # rhs for bias seeds
        nc.sync.dma_start(out=ones_s, in_=io["ones"][None, :SH])
        eps_sb = gconst.tile([1, 1], F32)
        nc.vector.memset(eps_sb, EPS)

        def load_w(pool, name):                   # (D, n) -> [128, DC, n] fp16
            w = io[name]
            t = pool.tile([P, w.shape[0] // P, w.shape[1]], DT16,
                          tag=f"w_{name}", name=f"w_{name}")
            nc.sync.dma_start(out=t, in_=w.rearrange("(c p) n -> p c n", p=P))
            return t

        p1w = root.enter_context(tc.tile_pool(name="p1w", bufs=1))
        wq_sb = load_w(p1w, "wq")
        bq_row = load_row("bq")
        bk_row = load_row("bk")
        bv_row = load_row("bv")

        # PE warmup: dummy matmuls on the first-arriving input keep the HAM
        # activity monitor busy so real matmuls start at 2.4 GHz
        with tc.tile_pool(name="warmps", bufs=1, space="PSUM") as warmps:
            wps = warmps.tile([1, SH], F32)
            for r in range(8):
                nc.tensor.matmul(wps, lhsT=xs16_sb[:, 0, 0:1], rhs=xs16_sb[:, 0, :],
                                 start=True, stop=True)

        wk_sb = load_w(p1w, "wk")
        wv_sb = load_w(p1w, "wv")

        attn_sb = gbig.tile([P, DC, SH], DT16)    # normalized attention out^T

        xs_sb = gbig.tile([P, DC, SH], DT)        # own x shard fp32 (residual)
        nc.sync.dma_start(out=xs_sb, in_=io["xsT"].rearrange("(c p) s -> p c s", p=P))
        bo_sb_t = gconst.tile([P, DC], F32, tag="bo_v", name="bo_v")
        nc.sync.dma_start(out=bo_sb_t, in_=io["bo"].rearrange("(c p) -> p c", p=P))
        b1_sb = gconst.tile([P, FC], F32, tag="b1_v", name="b1_v")
        nc.sync.dma_start(out=b1_sb, in_=io["b1"].rearrange("(c p) -> p c", p=P))
        b2_sb = gconst.tile([P, DC], F32, tag="b2_v", name="b2_v")
        nc.sync.dma_start(out=b2_sb, in_=io["b2"].rearrange("(c p) -> p c", p=P))
        g1_row, be1_row, g2_row, be2_row = (load_row(n) for n in ("g1", "be1", "g2", "be2"))

        # ======== attention era: kT/v/qT live here, freed before phase 3 ========
        with tc.tile_pool(name="abig", bufs=1) as abig, \
             tc.tile_pool(name="pp", bufs=1, space="PSUM") as pp, \
             tc.tile_pool(name="scp", bufs=1, space="PSUM") as scp, \
             tc.tile_pool(name="outp", bufs=1, space="PSUM") as outp, \
             tc.tile_pool(name="pm", bufs=2) as pm, \
             tc.tile_pool(name="p2a", bufs=2) as p2a, \
             tc.tile_pool(name="p2", bufs=2) as p2:

            kT_sb = abig.tile([P, DC, S], DT16)       # (x@wk)^T, full sequence
            qT_sb = abig.tile([P, DC, SH], DT16)      # (xs@wq)^T
            v_sb = abig.tile([P, TC, H, Dh + 1], BF16)  # v chunks + ones column
            nc.vector.memset(v_sb[:, :, :, Dh:Dh + 1], 1.0)

            def attend(heads, ti, m_t, out_ps):
                """scores + exp + mask + AV for one 128-key chunk, 4 heads."""
                for pr in range(2):
                    sc = scp.tile([P, HPC, SH], F32, tag="sc", name=f"sc_{heads[0]}_{ti}_{pr}")
                    for half in range(HPC):
                        h = heads[pr * HPC + half]
                        c = h // HPC
                        psl = ds(half * Dh, Dh)
                        nc.tensor.matmul(
                            sc[:, half, :], lhsT=kT_sb[psl, c, ds(ti * P, P)],
                            rhs=qT_sb[psl, c, :], start=True, stop=True,
                            tile_position=(half * Dh, 0))
                    a_t = p2a.tile([P, HPC, SH], BF16, tag=f"a{pr}", name=f"a{pr}")
                    nc.scalar.activation(out=a_t, in_=sc, func=AF.Exp)
                    am_t = p2a.tile([P, HPC, SH], BF16, tag=f"am{pr}", name=f"am{pr}")
                    for half in range(HPC):
                        nc.vector.tensor_mul(am_t[:, half, :], a_t[:, half, :], m_t)
                    for half in range(HPC):
                        h = heads[pr * HPC + half]
                        nc.tensor.matmul(out_ps[h], lhsT=v_sb[:, ti, h, :],
                                         rhs=am_t[:, half, :],
                                         start=(ti == 0), stop=(ti == TC - 1))

            def normalize(heads, out_ps):
                """fold 1/rowsum into out_ps, write fp16 attn_sb."""
                for j, h in enumerate(heads):
                    c, half = h // HPC, h % HPC
                    rec = p2.tile([1, SH], DT, tag="rec")
                    with nc.allow_low_precision(reason="fp32 storage"):
                        nc.vector.reciprocal(rec, out_ps[h][Dh:Dh + 1, :])
                    bc = scp.tile([Dh, SH], F32, tag="sc", name=f"bc{h}")
                    nc.tensor.matmul(bc, lhsT=ones_row[:1, :Dh], rhs=rec,
                                     start=True, stop=True)
                    bc_sb = p2.tile([Dh, SH], DT, tag="bcsb")
                    nc.scalar.copy(bc_sb, bc)
                    nc.vector.tensor_mul(attn_sb[ds(half * Dh, Dh), c, :],
                                         out_ps[h][0:Dh, :], bc_sb)

            heads0 = list(range(4))
            out_ps = {h: outp.tile([Dh + 1, SH], F32, tag=f"out{h % 4}", name=f"out_ps{h}")
                      for h in heads0}

            # ---- fused: kT/v production + group-0 attention ----
            with tc.tile_pool(name="p1x", bufs=2) as p1x:
                # q projection first (needs only xs16 + wq): seed bias, 4-chunk chain
                for co in range(DC):
                    ps = pp.tile([P, SH], F32, tag="kps", name="qps")
                    nc.tensor.matmul(ps, lhsT=bq_row[:, ds(co * P, P)], rhs=ones_s,
                                     start=True, stop=False)
                    for ci in range(DC):
                        nc.tensor.matmul(ps, lhsT=wq_sb[:, ci, ds(co * P, P)],
                                         rhs=xs16_sb[:, ci, :], start=False, stop=(ci == DC - 1))
                    nc.vector.tensor_copy(qT_sb[:, co, :], ps)

                for tb in range(TB):
                    xt = []
                    for ci in range(DC):
                        t = p1x.tile([P, 512], DT16, tag=f"xt{ci}", name=f"xt{tb}_{ci}")
                        nc.sync.dma_start(out=t, in_=xT[ds(ci * P, P), ts(tb, 512)])
                        xt.append(t)
                    m_ts = []
                    for tj in range(4):
                        ti = tb * 4 + tj
                        m_t = pm.tile([P, SH], BF16, tag="mask", name=f"m{ti}")
                        nc.sync.dma_start(out=m_t, in_=maskT[ds(ti * P, P), :])
                        m_ts.append(m_t)
                    # kT chunks 0,1 (group-0 needs these)
                    for co in range(2):
                        ps = pp.tile([P, 512], F32, tag="kps", name=f"kps{tb}_{co}")
                        nc.tensor.matmul(ps, lhsT=bk_row[:, ds(co * P, P)], rhs=ones_s,
                                         start=True, stop=False)
                        for ci in range(DC):
                            nc.tensor.matmul(ps, lhsT=wk_sb[:, ci, ds(co * P, P)],
                                             rhs=xt[ci], start=False, stop=(ci == DC - 1))
                        nc.vector.tensor_copy(kT_sb[:, co, ts(tb, 512)], ps)
                    # v for the block's 4 chunks
                    for tj in range(4):
                        ti = tb * 4 + tj
                        ps = pp.tile([P, D], F32, tag="vps", name=f"vps{ti}")
                        nc.tensor.matmul(ps, lhsT=ones_row, rhs=bv_row,
                                         start=True, stop=False)
                        for ci in range(DC):
                            nc.tensor.matmul(ps, lhsT=xt[ci][:, ds(tj * P, P)],
                                             rhs=wv_sb[:, ci, :], start=False, stop=(ci == DC - 1))
                        nc.vector.tensor_copy(v_sb[:, ti, :, 0:Dh],
                                              ps.rearrange("p (h d) -> p h d", h=H))
                    # group-0 attention on this block
                    for tj in range(4):
                        attend(heads0, tb * 4 + tj, m_ts[tj], out_ps)
                    # kT chunks 2,3 (group-1 only): PE stall filler
                    for co in range(2, DC):
                        ps = pp.tile([P, 512], F32, tag="kps", name=f"kps{tb}_{co}")
                        nc.tensor.matmul(ps, lhsT=bk_row[:, ds(co * P, P)], rhs=ones_s,
                                         start=True, stop=False)
                        for ci in range(DC):
                            nc.tensor.matmul(ps, lhsT=wk_sb[:, ci, ds(co * P, P)],
                                             rhs=xt[ci], start=False, stop=(ci == DC - 1))
                        nc.vector.tensor_copy(kT_sb[:, co, ts(tb, 512)], ps)

            normalize(heads0, out_ps)

            # ---- group-1 attention (from SBUF) + phase-3 weight prefetch ----
            wo_sb = load_w(p3w, "wo")
            w1_sb = load_w(p3w, "w1")

            heads1 = list(range(4, 8))
            out_ps2 = {h: outp.tile([Dh + 1, SH], F32, tag=f"out{h % 4}", name=f"o2_{h}")
                       for h in heads1}
            for ti in range(TC):
                m_t = pm.tile([P, SH], BF16, tag="mask", name=f"m2_{ti}")
                nc.sync.dma_start(out=m_t, in_=maskT[ds(ti * P, P), :])
                attend(heads1, ti, m_t, out_ps2)
            normalize(heads1, out_ps2)

        # ---------------- phase 3: out proj + LN1 + FFN + LN2 ----------------
        with tc.tile_pool(name="p3", bufs=2) as p3, \
             tc.tile_pool(name="p3big", bufs=1) as p3big, \
             tc.tile_pool(name="p3w2", bufs=2) as p3w2, \
             tc.tile_pool(name="p3ps", bufs=2, space="PSUM") as p3ps, \
             tc.tile_pool(name="p3st", bufs=1, space="PSUM") as p3st, \
             tc.tile_pool(name="p3bc", bufs=2, space="PSUM") as p3bc:

            def layernorm(src, g_row, be_row, dst):
                """src/dst: lists of DC [128, SH] tiles; stats over partitions."""
                mu_ps = p3st.tile([1, SH], F32, tag="mu")
                m2_ps = p3st.tile([1, SH], F32, tag="m2")
                for c in range(DC):
                    nc.tensor.matmul(mu_ps, lhsT=ones_col, rhs=src[c],
                                     start=(c == 0), stop=(c == DC - 1))
                for c in range(DC):
                    sq = p3.tile([P, SH], DT, tag="sq")
                    nc.scalar.activation(out=sq, in_=src[c], func=AF.Square)
                    nc.tensor.matmul(m2_ps, lhsT=ones_col, rhs=sq,
                                     start=(c == 0), stop=(c == DC - 1))
                mu_s = p3.tile([1, SH], DT, tag="mu_s")
                m2_s = p3.tile([1, SH], DT, tag="m2_s")
                nc.vector.tensor_scalar_mul(mu_s, mu_ps, -1.0 / D)  # negated mean
                nc.vector.tensor_scalar_mul(m2_s, m2_ps, 1.0 / D)
                var_s = p3.tile([1, SH], DT, tag="var_s")
                nc.vector.tensor_mul(var_s, mu_s, mu_s)
                nc.vector.tensor_sub(var_s, m2_s, var_s)
                rstd_s = p3.tile([1, SH], DT, tag="rstd_s")
                sd_s = p3.tile([1, SH], DT, tag="sd_s")
                nc.scalar.activation(out=sd_s, in_=var_s, func=AF.Sqrt, bias=eps_sb)
                with nc.allow_low_precision(reason="fp32 storage"):
                    nc.vector.reciprocal(rstd_s, sd_s)
                off_s = p3.tile([1, SH], DT, tag="off_s")
                nc.vector.tensor_mul(off_s, mu_s, rstd_s)
                # per-chunk fused affine: dst = x*(g x rstd) + (g x (-mu rstd) + be x 1)
                for c in range(DC):
                    sc_b = p3bc.tile([P, SH], F32, tag="sc_b")
                    of_b = p3bc.tile([P, SH], F32, tag="of_b")
                    nc.tensor.matmul(sc_b, lhsT=g_row[:, ds(c * P, P)], rhs=rstd_s,
                                     start=True, stop=True)
                    nc.tensor.matmul(of_b, lhsT=g_row[:, ds(c * P, P)], rhs=off_s,
                                     start=True, stop=False)
                    nc.tensor.matmul(of_b, lhsT=be_row[:, ds(c * P, P)], rhs=ones_s,
                                     start=False, stop=True)
                    t = p3.tile([P, SH], DT, tag="lnt")
                    nc.vector.tensor_mul(t, src[c], sc_b)
                    nc.vector.tensor_add(dst[c], t, of_b)

            # out projection + residual; bo pre-folded into the residual operand
            xr = [p3big.tile([P, SH], DT, tag=f"xr{c}", name=f"xr{c}") for c in range(DC)]
            xsb = [p3big.tile([P, SH], DT, tag=f"xsb{c}", name=f"xsb{c}") for c in range(DC)]
            for c in range(DC):
                nc.vector.tensor_scalar_add(xsb[c], xs_sb[:, c, :], bo_sb_t[:, c:c + 1])
            for co in range(DC):
                ps = p3ps.tile([P, SH], F32, tag="ps")
                for ci in range(DC):
                    nc.tensor.matmul(ps, lhsT=wo_sb[:, ci, ds(co * P, P)],
                                     rhs=attn_sb[:, ci, :],
                                     start=(ci == 0), stop=(ci == DC - 1))
                nc.vector.tensor_add(xr[co], ps, xsb[co])

            x1 = [p3big.tile([P, SH], DT, tag=f"x1{c}", name=f"x1{c}") for c in range(DC)]
            layernorm(xr, g1_row, be1_row, x1)
            x1h = [p3big.tile([P, SH], DT16, tag=f"x1h{c}", name=f"x1h{c}") for c in range(DC)]
            for c in range(DC):
                nc.vector.tensor_copy(x1h[c], x1[c])

            # FFN
            hT = p3big.tile([P, FC, SH], DT16, tag="hT")
            for fc in range(FC):
                ps = p3ps.tile([P, SH], F32, tag="ps")
                for ci in range(DC):
                    nc.tensor.matmul(ps, lhsT=w1_sb[:, ci, ds(fc * P, P)], rhs=x1h[ci],
                                     start=(ci == 0), stop=(ci == DC - 1))
                nc.scalar.activation(out=hT[:, fc, :], in_=ps, func=AF.Relu,
                                     bias=b1_sb[:, fc:fc + 1])
            xr2 = [p3big.tile([P, SH], DT, tag=f"xr2{c}", name=f"xr2{c}") for c in range(DC)]
            x1b = [p3big.tile([P, SH], DT, tag=f"x1b{c}", name=f"x1b{c}") for c in range(DC)]
            for c in range(DC):
                nc.vector.tensor_scalar_add(x1b[c], x1[c], b2_sb[:, c:c + 1])
            w2_v = io["w2"].rearrange("(c p) n -> p c n", p=P)
            for co in range(DC):
                wt2 = p3w2.tile([P, FC, P], DT16, tag="wt2")
                nc.sync.dma_start(out=wt2, in_=w2_v[:, :, ds(co * P, P)])
                ps = p3ps.tile([P, SH], F32, tag="ps")
                for fc in range(FC):
                    nc.tensor.matmul(ps, lhsT=wt2[:, fc, :], rhs=hT[:, fc, :],
                                     start=(fc == 0), stop=(fc == FC - 1))
                nc.vector.tensor_add(xr2[co], ps, x1b[co])

            x2 = [p3big.tile([P, SH], F32, tag=f"x2{c}", name=f"x2{c}") for c in range(DC)]
            layernorm(xr2, g2_row, be2_row, x2)
            for c in range(DC):
                nc.sync.dma_start(out=outT[ds(c * P, P), :], in_=x2[c])


# ---------------------------------------------------------------------------
# host-side entry point
# ---------------------------------------------------------------------------

_CACHE = {}


def _get_compiled(S, D, F, H):
    key = (S, D, F, H)
    if key not in _CACHE:
        nc = bacc.Bacc("TRN2", target_bir_lowering=False, debug=False,
                       num_devices=N_CORES)
        build_encoder_kernel(nc, S=S, D=D, F=F, H=H, n_cores=N_CORES)
        nc.compile()
        _CACHE[key] = nc
    return _CACHE[key]


def make_in_maps(x, mask, weights, S, D, n_cores=N_CORES):
    """Shard + lay out inputs per core. x: (S, D) f32; mask: (S, S) int."""
    SH = S // n_cores
    xT = np.ascontiguousarray(x.T)                       # (D, S)
    maskb = (mask != 0)
    in_maps = []
    for c in range(n_cores):
        sl = slice(c * SH, (c + 1) * SH)
        im = {
            "xT": xT.astype(np.float16),
            "xsT": np.ascontiguousarray(xT[:, sl]),
            "xs16": np.ascontiguousarray(xT[:, sl]).astype(np.float16),
            "maskT": np.ascontiguousarray(maskb[sl, :].T).astype(ml_dtypes.bfloat16),
            "ones": np.ones(512, np.float32),
        }
        im.update({k: (v.astype(np.float16) if k in ("wq", "wk", "wv", "wo", "w1", "w2")
                       else v) for k, v in weights.items()})
        in_maps.append(im)
    return in_maps


def kernel(**inputs):
    x = np.asarray(inputs["x"], np.float32)
    mask = np.asarray(inputs["mask"])
    B, S, D = x.shape
    F = inputs["w1"].shape[1]
    H = 8
    assert B == 1
    weights = {k: np.asarray(inputs[k], np.float32)
               for k in ("wq", "wk", "wv", "wo", "w1", "w2",
                         "bq", "bk", "bv", "bo", "b1", "b2",
                         "g1", "be1", "g2", "be2")}
    nc = _get_compiled(S, D, F, H)
    in_maps = make_in_maps(x[0], mask, weights, S, D)
    res = run_bass_kernel_spmd(nc, in_maps, list(range(N_CORES)))
    SH = S // N_CORES
    out = np.empty((S, D), np.float32)
    for c in range(N_CORES):
        out[c * SH:(c + 1) * SH, :] = res.results[c]["outT"].T
    return out[None]


# revision 7
# speedup vs baseline: 1.0702x; 1.0702x over previous
"""Trainium2 Bass kernel for a dense transformer encoder layer.

Problem: B=1, S=4096, D=512, F=2048, H=8 heads (Dh=64), fp32 reference,
attention WITHOUT 1/sqrt(Dh) scaling, int mask (0 -> -1e9 before softmax),
two LayerNorms, ReLU FFN.

Sharding (query/row-parallel, no collectives): every core redundantly
computes the full kT = (x@wk).T and v = x@wv, plus its own 512-query
shard. Each core computes attention + output projection + LN + FFN + LN
for its queries and writes outT (D, 512); the host transposes and
concatenates the shards.

v2 structure (fused pipeline, all engines overlapped):
  - The kT/v production loop (8 t-blocks of 512) is FUSED with group-0
    attention (heads 0-3): as soon as a 512-key block's kT (feature
    chunks 0,1) and v land in SBUF, the scores/exp/mask/AV for its four
    128-key chunks run. kT chunks 2,3 (only needed by group 1) are
    emitted last in each block as PE stall-filler, so the PE never idles
    long enough for the HAM clock monitor to re-throttle it to 1.2 GHz.
  - ALL projection biases (bq/bk/bv) are folded in as K=1 seed matmuls
    into PSUM before the accumulation chain (216ns each on the PE), and
    PSUM->SBUF evacuation is done by DVE tensor_copy at 2x rate. The ACT
    engine does nothing but exp (its 1 elem/cycle/lane @1.2GHz is the
    attention-phase floor) + the two LN sqrt calls.
  - PSUM budget in the fused phase: 4 banks out_ps (heads 0-3) + 2 banks
    scores + 1 bank kT chain + 1 bank v chain = 8 exactly. Group 1 runs
    afterwards from SBUF with double-buffered score PSUM (ACT ~100%
    duty) while the wo/w1/w2 weights for phase 3 prefetch over DMA.
  - Phase 3 (out-proj + LN1 + FFN + LN2) then runs with zero DMA waits.

Softmax skips max-subtraction (|scores| < ~60 fits bf16 range); the
ones-column in v yields denominators for free; per-query 1/sum is folded
in via a K=1 broadcast matmul. LayerNorm runs transposed: partition-dim
statistics via ones-vector matmuls, per-column stats broadcast with K=1
outer products, gamma/beta folded into the broadcast.

dtypes: fp16 (10-bit mantissa) for QKV projections, K/Q storage, scores,
FFN/out-projection weights+activations; fp32r for K=1 broadcast matmuls
and LN/residual arithmetic; bf16 for exp outputs / V / mask; fp32
accumulation in PSUM.
"""

import numpy as np
import ml_dtypes

import concourse.bass as bass
import concourse.bacc as bacc
import concourse.tile as tile
from concourse import mybir
from concourse.bass import ts, ds
from concourse.bass_utils import run_bass_kernel_spmd

AF = mybir.ActivationFunctionType
F32 = mybir.dt.float32
DT = mybir.dt.float32r  # fp32 storage, single-pass PE mode
DT16 = mybir.dt.float16
BF16 = mybir.dt.bfloat16

N_CORES = 8
EPS = 1e-5


def build_encoder_kernel(nc, S=4096, D=512, F=2048, H=8, n_cores=8):
    """Emit the SPMD per-core program. Returns nothing (declares DRAM I/O)."""
    P = 128
    SH = S // n_cores          # query shard per core
    DC = D // P                # feature chunks of 128
    FC = F // P                # ffn chunks of 128
    TB = S // 512              # 512-wide t blocks
    TC = S // P                # 128-tall t chunks
    Dh = D // H
    assert Dh == 64 and DC * P == D and SH % 2 == 0

    d = lambda name, shape, dt: nc.dram_tensor(name, shape, dt, kind="ExternalInput").ap()
    xT = d("xT", [D, S], DT16)
    xsT = d("xsT", [D, SH], DT)
    xs16 = d("xs16", [D, SH], DT16)
    maskT = d("maskT", [S, SH], BF16)
    wq, wk, wv, wo = (d(n, [D, D], DT16) for n in ("wq", "wk", "wv", "wo"))
    w1 = d("w1", [D, F], DT16)
    w2 = d("w2", [F, D], DT16)
    bq, bk = (d(n, [D], F32) for n in ("bq", "bk"))
    bv = d("bv", [D], DT)
    bo = d("bo", [D], F32)
    b1 = d("b1", [F], F32)
    b2 = d("b2", [D], F32)
    g1, be1, g2, be2 = (d(n, [D], DT) for n in ("g1", "be1", "g2", "be2"))
    ones = d("ones", [512], DT)
    outT = nc.dram_tensor("outT", [D, SH], F32, kind="ExternalOutput").ap()

    with tile.TileContext(nc) as tc:
        _emit(nc, tc, locals())


def _emit(nc, tc, io):
    P = 128
    xT, maskT, outT = io["xT"], io["maskT"], io["outT"]
    S, D, F, H = io["S"], io["D"], io["F"], io["H"]
    SH, DC, FC, TB, TC, Dh = io["SH"], io["DC"], io["FC"], io["TB"], io["TC"], io["Dh"]
    HPC = P // Dh              # heads per 128-feature chunk (2)

    from contextlib import ExitStack
    with ExitStack() as root:
        gconst = root.enter_context(tc.tile_pool(name="gconst", bufs=1))
        gbig = root.enter_context(tc.tile_pool(name="gbig", bufs=1))
        p3w = root.enter_context(tc.tile_pool(name="p3w", bufs=1))

        # ---- startup DMAs, highest priority first ----
        xs16_sb = gbig.tile([P, DC, SH], DT16)    # own x shard fp16 (q proj rhs)
        nc.sync.dma_start(out=xs16_sb, in_=io["xs16"].rearrange("(c p) s -> p c s", p=P))

        def load_row(name, dt=DT):                # (n,) -> [1, n] row
            t = gconst.tile([1, io[name].shape[0]], dt, tag=f"row_{name}", name=f"row_{name}")
            nc.sync.dma_start(out=t, in_=io[name][None, :])
            return t

        ones_row = gconst.tile([1, P], DT)        # lhsT for K=1 broadcasts
        nc.sync.dma_start(out=ones_row, in_=io["ones"][None, :P])
        ones_col = gconst.tile([P, 1], DT)        # lhsT for partition sums
        nc.sync.dma_start(out=ones_col, in_=io["ones"][:P, None])
        ones_s = gconst.tile([1, SH], DT)         # rhs for bias seeds
        nc.sync.dma_start(out=ones_s, in_=io["ones"][None, :SH])
        eps_sb = gconst.tile([1, 1], F32)
        nc.vector.memset(eps_sb, EPS)

        def load_w(pool, name):                   # (D, n) -> [128, DC, n] fp16
            w = io[name]
            t = pool.tile([P, w.shape[0] // P, w.shape[1]], DT16,
                          tag=f"w_{name}", name=f"w_{name}")
            nc.sync.dma_start(out=t, in_=w.rearrange("(c p) n -> p c n", p=P))
            return t

        p1w = root.enter_context(tc.tile_pool(name="p1w", bufs=1))
        wq_sb = load_w(p1w, "wq")

        def load_vec(name, chunks):               # (n,) -> [128, chunks]
            t = gconst.tile([P, chunks], F32, tag=f"vec_{name}", name=f"vec_{name}")
            nc.sync.dma_start(out=t, in_=io[name].rearrange("(c p) -> p c", p=P))
            return t

        bq_sb, bk_sb = load_vec("bq", DC), load_vec("bk", DC)
        bv_row = load_row("bv")

        # PE warmup: dummy matmuls on the first-arriving input keep the HAM
        # activity monitor busy so real matmuls start at 2.4 GHz
        with tc.tile_pool(name="warmps", bufs=1, space="PSUM") as warmps:
            wps = warmps.tile([1, SH], F32)
            for r in range(16):
                nc.tensor.matmul(wps, lhsT=xs16_sb[:, 0, 0:1], rhs=xs16_sb[:, 0, :],
                                 start=True, stop=True)

        wk_sb = load_w(p1w, "wk")
        wv_sb = load_w(p1w, "wv")

        attn_sb = gbig.tile([P, DC, SH], DT16)    # normalized attention out^T

        xs_sb = gbig.tile([P, DC, SH], DT)        # own x shard fp32 (residual)
        nc.sync.dma_start(out=xs_sb, in_=io["xsT"].rearrange("(c p) s -> p c s", p=P))
        bo_sb_t = gconst.tile([P, DC], F32, tag="bo_v", name="bo_v")
        nc.sync.dma_start(out=bo_sb_t, in_=io["bo"].rearrange("(c p) -> p c", p=P))
        b1_sb = gconst.tile([P, FC], F32, tag="b1_v", name="b1_v")
        nc.sync.dma_start(out=b1_sb, in_=io["b1"].rearrange("(c p) -> p c", p=P))
        b2_sb = gconst.tile([P, DC], F32, tag="b2_v", name="b2_v")
        nc.sync.dma_start(out=b2_sb, in_=io["b2"].rearrange("(c p) -> p c", p=P))
        g1_row, be1_row, g2_row, be2_row = (load_row(n) for n in ("g1", "be1", "g2", "be2"))

        # ======== attention era: kT/v/qT live here, freed before phase 3 ========
        with tc.tile_pool(name="abig", bufs=1) as abig, \
             tc.tile_pool(name="scp", bufs=1, space="PSUM") as scp, \
             tc.tile_pool(name="pm", bufs=2) as pm, \
             tc.tile_pool(name="p2a", bufs=2) as p2a, \
             tc.tile_pool(name="p2", bufs=2) as p2:

            kT_sb = abig.tile([P, DC, S], DT16)       # (x@wk)^T, full sequence
            qT_sb = abig.tile([P, DC, SH], DT16)      # (xs@wq)^T
            v_sb = abig.tile([P, TC, H, Dh + 1], BF16)  # v chunks + ones column
            nc.vector.memset(v_sb[:, :, :, Dh:Dh + 1], 1.0)
            bvb_sb = abig.tile([P, D], F32)           # bv broadcast across partitions

            def attend(heads, ti, m_t, out_ps, sc_pools):
                """scores + exp + mask + AV for one 128-key chunk, len(heads) heads."""
                for pr in range(len(heads) // HPC):
                    pool, tag = sc_pools[(ti + pr) % len(sc_pools)]
                    sc = pool.tile([P, HPC, SH], F32, tag=tag, name=f"sc_{heads[0]}_{ti}_{pr}")
                    for half in range(HPC):
                        h = heads[pr * HPC + half]
                        c = h // HPC
                        psl = ds(half * Dh, Dh)
                        nc.tensor.matmul(
                            sc[:, half, :], lhsT=kT_sb[psl, c, ds(ti * P, P)],
                            rhs=qT_sb[psl, c, :], start=True, stop=True,
                            tile_position=(half * Dh, 0))
                    a_t = p2a.tile([P, HPC, SH], BF16, tag=f"a{pr}", name=f"a{pr}")
                    nc.scalar.activation(out=a_t, in_=sc, func=AF.Exp)
                    am_t = p2a.tile([P, HPC, SH], BF16, tag=f"am{pr}", name=f"am{pr}")
                    for half in range(HPC):
                        nc.vector.tensor_mul(am_t[:, half, :], a_t[:, half, :], m_t)
                    for half in range(HPC):
                        h = heads[pr * HPC + half]
                        nc.tensor.matmul(out_ps[h], lhsT=v_sb[:, ti, h, :],
                                         rhs=am_t[:, half, :],
                                         start=(ti == 0), stop=(ti == TC - 1))

            def normalize(h, out_ps):
                """fold 1/rowsum into out_ps -> fp16 attn_sb; DVE+GpSimd only."""
                c, half = h // HPC, h % HPC
                rec = p2.tile([1, SH], DT, tag="rec")
                with nc.allow_low_precision(reason="fp32 storage"):
                    nc.vector.reciprocal(rec, out_ps[h][Dh:Dh + 1, :])
                bc_sb = p2.tile([Dh, SH], DT, tag="bcsb")
                nc.gpsimd.partition_broadcast(bc_sb, rec, channels=Dh)
                nc.vector.tensor_mul(attn_sb[ds(half * Dh, Dh), c, :],
                                     out_ps[h][0:Dh, :], bc_sb)

            def mask_tile(ti, name):
                m_t = pm.tile([P, SH], BF16, tag="mask", name=name)
                nc.sync.dma_start(out=m_t, in_=maskT[ds(ti * P, P), :])
                return m_t

            with tc.tile_pool(name="outp", bufs=1, space="PSUM") as outp:
                heads0 = list(range(4))
                out_ps = {h: outp.tile([Dh + 1, SH], F32, tag=f"out{h}", name=f"out_ps{h}")
                          for h in heads0}

                # ---- fused: kT/v production + group-0 attention ----
                with tc.tile_pool(name="pp", bufs=1, space="PSUM") as pp, \
                     tc.tile_pool(name="p1x", bufs=2) as p1x:
                    # bv broadcast: bvb[p, do] = bv[do]
                    bv_ps = pp.tile([P, D], F32, tag="vps", name="bv_ps")
                    nc.tensor.matmul(bv_ps, lhsT=ones_row, rhs=bv_row,
                                     start=True, stop=True)
                    nc.vector.tensor_copy(bvb_sb, bv_ps)
                    # q projection (needs only xs16 + wq)
                    for co in range(DC):
                        ps = pp.tile([P, SH], F32, tag="kps", name="qps")
                        for ci in range(DC):
                            nc.tensor.matmul(ps, lhsT=wq_sb[:, ci, ds(co * P, P)],
                                             rhs=xs16_sb[:, ci, :],
                                             start=(ci == 0), stop=(ci == DC - 1))
                        nc.scalar.activation(out=qT_sb[:, co, :], in_=ps,
                                             func=AF.Identity, bias=bq_sb[:, co:co + 1])

                    for tb in range(TB):
                        xt = []
                        for ci in range(DC):
                            t = p1x.tile([P, 512], DT16, tag=f"xt{ci}", name=f"xt{tb}_{ci}")
                            nc.sync.dma_start(out=t, in_=xT[ds(ci * P, P), ts(tb, 512)])
                            xt.append(t)
                        m_ts = [mask_tile(tb * 4 + tj, f"m{tb * 4 + tj}") for tj in range(4)]
                        # kT chunks 0,1 (group 0 needs these now)
                        for co in range(2):
                            ps = pp.tile([P, 512], F32, tag="kps", name=f"kps{tb}_{co}")
                            for ci in range(DC):
                                nc.tensor.matmul(ps, lhsT=wk_sb[:, ci, ds(co * P, P)],
                                                 rhs=xt[ci], start=(ci == 0), stop=(ci == DC - 1))
                            nc.scalar.activation(out=kT_sb[:, co, ts(tb, 512)], in_=ps,
                                                 func=AF.Identity, bias=bk_sb[:, co:co + 1])
                        # v for the block's 4 chunks
                        for tj in range(4):
                            ti = tb * 4 + tj
                            ps = pp.tile([P, D], F32, tag="vps", name=f"vps{ti}")
                            for ci in range(DC):
                                nc.tensor.matmul(ps, lhsT=xt[ci][:, ds(tj * P, P)],
                                                 rhs=wv_sb[:, ci, :],
                                                 start=(ci == 0), stop=(ci == DC - 1))
                            nc.vector.tensor_add(
                                out=v_sb[:, ti, :, 0:Dh],
                                in0=ps.rearrange("p (h d) -> p h d", h=H),
                                in1=bvb_sb.rearrange("p (h d) -> p h d", h=H))
                        # group-0 attention on this block
                        for tj in range(4):
                            attend(heads0, tb * 4 + tj, m_ts[tj], out_ps,
                                   [(scp, "sc")])
                        # kT chunks 2,3 (group-1 only): PE stall filler
                        for co in range(2, DC):
                            ps = pp.tile([P, 512], F32, tag="kps", name=f"kps{tb}_{co}")
                            for ci in range(DC):
                                nc.tensor.matmul(ps, lhsT=wk_sb[:, ci, ds(co * P, P)],
                                                 rhs=xt[ci], start=(ci == 0), stop=(ci == DC - 1))
                            nc.scalar.activation(out=kT_sb[:, co, ts(tb, 512)], in_=ps,
                                                 func=AF.Identity, bias=bk_sb[:, co:co + 1])

                for h in heads0:
                    normalize(h, out_ps)

                # ---- pass A: heads 4,5 (out banks reuse the freed kps/vps) ----
                wo_sb = load_w(p3w, "wo")
                w1_sb = load_w(p3w, "w1")
                with tc.tile_pool(name="ppA", bufs=1, space="PSUM") as ppA:
                    headsA = [4, 5]
                    out_psA = {h: ppA.tile([Dh + 1, SH], F32, tag=f"oa{h}", name=f"oa{h}")
                               for h in headsA}
                    for ti in range(TC):
                        m_t = mask_tile(ti, f"mA_{ti}")
                        attend(headsA, ti, m_t, out_psA, [(scp, "sc")])
                    for h in headsA:
                        normalize(h, out_psA)

            # ---- pass B: heads 6,7 (outp closed; double-buffered scores) ----
            with tc.tile_pool(name="ppB", bufs=1, space="PSUM") as ppB:
                headsB = [6, 7]
                out_psB = {h: ppB.tile([Dh + 1, SH], F32, tag=f"ob{h}", name=f"ob{h}")
                           for h in headsB}
                for ti in range(TC):
                    m_t = mask_tile(ti, f"mB_{ti}")
                    attend(headsB, ti, m_t, out_psB, [(scp, "sc"), (ppB, "sc2")])
                for h in headsB:
                    normalize(h, out_psB)

        # ---------------- phase 3: out proj + LN1 + FFN + LN2 ----------------
        with tc.tile_pool(name="p3", bufs=2) as p3, \
             tc.tile_pool(name="p3big", bufs=1) as p3big, \
             tc.tile_pool(name="p3w2", bufs=2) as p3w2, \
             tc.tile_pool(name="p3ps", bufs=2, space="PSUM") as p3ps, \
             tc.tile_pool(name="p3st", bufs=1, space="PSUM") as p3st, \
             tc.tile_pool(name="p3bc", bufs=2, space="PSUM") as p3bc:

            def layernorm(src, g_row, be_row, dst):
                """src/dst: lists of DC [128, SH] tiles; stats over partitions."""
                mu_ps = p3st.tile([1, SH], F32, tag="mu")
                m2_ps = p3st.tile([1, SH], F32, tag="m2")
                for c in range(DC):
                    nc.tensor.matmul(mu_ps, lhsT=ones_col, rhs=src[c],
                                     start=(c == 0), stop=(c == DC - 1))
                for c in range(DC):
                    sq = p3.tile([P, SH], DT, tag="sq")
                    nc.scalar.activation(out=sq, in_=src[c], func=AF.Square)
                    nc.tensor.matmul(m2_ps, lhsT=ones_col, rhs=sq,
                                     start=(c == 0), stop=(c == DC - 1))
                mu_s = p3.tile([1, SH], DT, tag="mu_s")
                m2_s = p3.tile([1, SH], DT, tag="m2_s")
                nc.vector.tensor_scalar_mul(mu_s, mu_ps, -1.0 / D)  # negated mean
                nc.vector.tensor_scalar_mul(m2_s, m2_ps, 1.0 / D)
                var_s = p3.tile([1, SH], DT, tag="var_s")
                nc.vector.tensor_mul(var_s, mu_s, mu_s)
                nc.vector.tensor_sub(var_s, m2_s, var_s)
                rstd_s = p3.tile([1, SH], DT, tag="rstd_s")
                sd_s = p3.tile([1, SH], DT, tag="sd_s")
                nc.scalar.activation(out=sd_s, in_=var_s, func=AF.Sqrt, bias=eps_sb)
                with nc.allow_low_precision(reason="fp32 storage"):
                    nc.vector.reciprocal(rstd_s, sd_s)
                off_s = p3.tile([1, SH], DT, tag="off_s")
                nc.vector.tensor_mul(off_s, mu_s, rstd_s)
                # per-chunk fused affine: dst = x*(g x rstd) + (g x (-mu rstd) + be x 1)
                for c in range(DC):
                    sc_b = p3bc.tile([P, SH], F32, tag="sc_b")
                    of_b = p3bc.tile([P, SH], F32, tag="of_b")
                    nc.tensor.matmul(sc_b, lhsT=g_row[:, ds(c * P, P)], rhs=rstd_s,
                                     start=True, stop=True)
                    nc.tensor.matmul(of_b, lhsT=g_row[:, ds(c * P, P)], rhs=off_s,
                                     start=True, stop=False)
                    nc.tensor.matmul(of_b, lhsT=be_row[:, ds(c * P, P)], rhs=ones_s,
                                     start=False, stop=True)
                    t = p3.tile([P, SH], DT, tag="lnt")
                    nc.vector.tensor_mul(t, src[c], sc_b)
                    nc.vector.tensor_add(dst[c], t, of_b)

            # out projection + residual; bo pre-folded into the residual operand
            xr = [p3big.tile([P, SH], DT, tag=f"xr{c}", name=f"xr{c}") for c in range(DC)]
            xsb = [p3big.tile([P, SH], DT, tag=f"xsb{c}", name=f"xsb{c}") for c in range(DC)]
            for c in range(DC):
                nc.vector.tensor_scalar_add(xsb[c], xs_sb[:, c, :], bo_sb_t[:, c:c + 1])
            for co in range(DC):
                ps = p3ps.tile([P, SH], F32, tag="ps")
                for ci in range(DC):
                    nc.tensor.matmul(ps, lhsT=wo_sb[:, ci, ds(co * P, P)],
                                     rhs=attn_sb[:, ci, :],
                                     start=(ci == 0), stop=(ci == DC - 1))
                nc.vector.tensor_add(xr[co], ps, xsb[co])

            x1 = [p3big.tile([P, SH], DT, tag=f"x1{c}", name=f"x1{c}") for c in range(DC)]
            layernorm(xr, g1_row, be1_row, x1)
            x1h = [p3big.tile([P, SH], DT16, tag=f"x1h{c}", name=f"x1h{c}") for c in range(DC)]
            for c in range(DC):
                nc.vector.tensor_copy(x1h[c], x1[c])

            # FFN
            hT = p3big.tile([P, FC, SH], DT16, tag="hT")
            for fc in range(FC):
                ps = p3ps.tile([P, SH], F32, tag="ps")
                for ci in range(DC):
                    nc.tensor.matmul(ps, lhsT=w1_sb[:, ci, ds(fc * P, P)], rhs=x1h[ci],
                                     start=(ci == 0), stop=(ci == DC - 1))
                nc.scalar.activation(out=hT[:, fc, :], in_=ps, func=AF.Relu,
                                     bias=b1_sb[:, fc:fc + 1])
            xr2 = [p3big.tile([P, SH], DT, tag=f"xr2{c}", name=f"xr2{c}") for c in range(DC)]
            x1b = [p3big.tile([P, SH], DT, tag=f"x1b{c}", name=f"x1b{c}") for c in range(DC)]
            for c in range(DC):
                nc.vector.tensor_scalar_add(x1b[c], x1[c], b2_sb[:, c:c + 1])
            w2_v = io["w2"].rearrange("(c p) n -> p c n", p=P)
            for co in range(DC):
                wt2 = p3w2.tile([P, FC, P], DT16, tag="wt2")
                nc.sync.dma_start(out=wt2, in_=w2_v[:, :, ds(co * P, P)])
                ps = p3ps.tile([P, SH], F32, tag="ps")
                for fc in range(FC):
                    nc.tensor.matmul(ps, lhsT=wt2[:, fc, :], rhs=hT[:, fc, :],
                                     start=(fc == 0), stop=(fc == FC - 1))
                nc.vector.tensor_add(xr2[co], ps, x1b[co])

            x2 = [p3big.tile([P, SH], F32, tag=f"x2{c}", name=f"x2{c}") for c in range(DC)]
            layernorm(xr2, g2_row, be2_row, x2)
            for c in range(DC):
                nc.sync.dma_start(out=outT[ds(c * P, P), :], in_=x2[c])


# ---------------------------------------------------------------------------
# host-side entry point
# ---------------------------------------------------------------------------

_CACHE = {}


def _get_compiled(S, D, F, H):
    key = (S, D, F, H)
    if key not in _CACHE:
        nc = bacc.Bacc("TRN2", target_bir_lowering=False, debug=False,
                       num_devices=N_CORES)
        build_encoder_kernel(nc, S=S, D=D, F=F, H=H, n_cores=N_CORES)
        nc.compile()
        _CACHE[key] = nc
    return _CACHE[key]


def make_in_maps(x, mask, weights, S, D, n_cores=N_CORES):
    """Shard + lay out inputs per core. x: (S, D) f32; mask: (S, S) int."""
    SH = S // n_cores
    xT = np.ascontiguousarray(x.T)                       # (D, S)
    maskb = (mask != 0)
    in_maps = []
    for c in range(n_cores):
        sl = slice(c * SH, (c + 1) * SH)
        im = {
            "xT": xT.astype(np.float16),
            "xsT": np.ascontiguousarray(xT[:, sl]),
            "xs16": np.ascontiguousarray(xT[:, sl]).astype(np.float16),
            "maskT": np.ascontiguousarray(maskb[sl, :].T).astype(ml_dtypes.bfloat16),
            "ones": np.ones(512, np.float32),
        }
        im.update({k: (v.astype(np.float16) if k in ("wq", "wk", "wv", "wo", "w1", "w2")
                       else v) for k, v in weights.items()})
        in_maps.append(im)
    return in_maps


def kernel(**inputs):
    x = np.asarray(inputs["x"], np.float32)
    mask = np.asarray(inputs["mask"])
    B, S, D = x.shape
    F = inputs["w1"].shape[1]
    H = 8
    assert B == 1
    weights = {k: np.asarray(inputs[k], np.float32)
               for k in ("wq", "wk", "wv", "wo", "w1", "w2",
                         "bq", "bk", "bv", "bo", "b1", "b2",
                         "g1", "be1", "g2", "be2")}
    nc = _get_compiled(S, D, F, H)
    in_maps = make_in_maps(x[0], mask, weights, S, D)
    res = run_bass_kernel_spmd(nc, in_maps, list(range(N_CORES)))
    SH = S // N_CORES
    out = np.empty((S, D), np.float32)
    for c in range(N_CORES):
        out[c * SH:(c + 1) * SH, :] = res.results[c]["outT"].T
    return out[None]
